# revision 3
# baseline (speedup 1.0000x reference)
"""EquivariantAttention kernel for 8 trn2 NeuronCores (Bass/Tile) — v2.

Strategy (edges sharded by destination node; host sorts by dst):
  Core c owns ~10 windows of 128 consecutive nodes and all edges into them,
  so edge-softmax and the scatter-sum are core-local.

Device pipeline per 128-edge tile, rebalanced across engines:
  PE  : MLP1 (fp32r, tile-pairs), MLP2 (fp32r, 384-col halves), and the
        one-hot segment matmuls that now ALSO perform the v-path m-reduction
        (seg psum accumulates un-reduced (c',m,d) products; the 32-way m-sum
        happens once per 128-node window instead of once per edge).
  ACT : relu+cast h, rw psum->sbuf bf16 casts, exp.
  DVE : bf16 2x-mode products (fe*basis, rw*tmp), bf16 tree-reduce for a
        slice of the k/q m-reduction, score dot, leaky-relu (fused
        scalar_tensor_tensor), fused one-hot (tensor_scalar is_equal*ex,
        4x mode) which folds the softmax numerator scaling into the
        segment matmul weights.
  Pool: fe*basis k-reduce, the other slice of the k/q m-reduction, and the
        per-window (c',m,d)->(c',d) flush reduction.
Softmax: out[n] = segsum(ex*v)[n] / segsum(ex)[n] (max-subtraction skipped,
  scores bounded; division folded into one per-node reciprocal).
"""

import time

import numpy as np

import concourse.bacc as bacc
import concourse.bass as bass
import concourse.mybir as mybir
import concourse.tile as tile
from concourse.bass_utils import run_bass_kernel_spmd

F32 = mybir.dt.float32
FP16 = mybir.dt.float16
AF = mybir.ActivationFunctionType
ALU = mybir.AluOpType
AX = mybir.AxisListType

E = 160000
N = 10000
NC = 8
WIN = 128
NWIN_G = (N + WIN - 1) // WIN      # 79 global windows
M1, M2, D1, D2, NREPS = 16, 8, 3, 3, 2
EDGE_DIM, HID, NHEADS = 32, 64, 4
HIDDEN = M2 * D2                   # 24
TEMP = float(HIDDEN) ** (-0.5)
CKQ = 16                           # k,q channels
CPOOL = 2                          # k/q channels computed on Pool (rest: DVE)
PROD_POOL = True                   # fe*basis products on Pool engine
PKW = 68                           # pk columns: 48 fe + 18 basis + 2 nid
SEGW = 2 * 193                     # per-part psum cols: 2 heads x (den + 192)

_CACHE = {}
LAST_RUN_S = None


def _build(T, toff, nwin):
    """toff[w] = first tile of window-slot w (len nwin+1); tiles inside a
    window never cross window boundaries."""
    nc = bacc.Bacc(None, target_bir_lowering=False, debug=False)
    EP = T * 128
    ef_d = nc.dram_tensor("efT", [EDGE_DIM + 1, EP], FP16, kind="ExternalInput")
    pk_d = nc.dram_tensor("packed", [T, 128, PKW], FP16, kind="ExternalInput")
    w1_d = nc.dram_tensor("w1", [EDGE_DIM + 1, HID], FP16, kind="ExternalInput")
    w2_d = nc.dram_tensor("w2", [HID + 1, 768], FP16, kind="ExternalInput")
    io_d = nc.dram_tensor("iota", [128, 128], FP16, kind="ExternalInput")
    out_d = nc.dram_tensor("out", [nwin * 128, HIDDEN], F32, kind="ExternalOutput")

    with tile.TileContext(nc) as tc:
        with (
            tc.tile_pool(name="const", bufs=1) as cp,
            tc.tile_pool(name="sb", bufs=3) as pool,
            tc.tile_pool(name="sm", bufs=2) as sm,
            tc.tile_pool(name="hps", bufs=1, space="PSUM") as hp,
            tc.tile_pool(name="rwps", bufs=1, space="PSUM") as rp,
            tc.tile_pool(name="seg", bufs=2, space="PSUM") as sp,
            nc.allow_low_precision(reason="bf16 pipeline; tolerance 2e-2"),
        ):
            w1_sb = cp.tile([EDGE_DIM + 1, HID], FP16)
            nc.sync.dma_start(w1_sb[:], w1_d[:])
            w2_sb = cp.tile([HID + 1, 768], FP16)
            nc.sync.dma_start(w2_sb[:], w2_d[:])
            io_sb = cp.tile([128, 128], FP16)
            nc.sync.dma_start(io_sb[:], io_d[:])
            ones_sb = cp.tile([128, 1], FP16)
            nc.vector.memset(ones_sb[:], 1.0)
            nbias = cp.tile([128, 1], F32)
            nc.vector.memset(nbias[:], -4.0)
            # manual rotation for h so the ones-row is set once
            h_bufs = [cp.tile([HID + 1, 256], FP16, name=f"hbuf{i}") for i in range(3)]
            for hb in h_bufs:
                nc.vector.memset(hb[HID : HID + 1, :], 1.0)

            for w in range(nwin):
                t0, t1 = toff[w], toff[w + 1]
                seg0 = sp.tile([128, SEGW], F32, tag="seg0", name="seg0")
                seg1 = sp.tile([128, SEGW], F32, tag="seg1", name="seg1")
                segs = (seg0, seg1)
                nc.scalar.memzero(seg0[:])
                nc.scalar.memzero(seg1[:])
                for t in range(t0, t1):
                    # --- MLP1 on global tile pairs (256 cols -> fp32r rate) ---
                    hb = h_bufs[(t // 2) % 3]
                    hoff = (t % 2) * 128
                    if t % 2 == 0:
                        ncols = min(256, (T - t) * 128)
                        h_ps = hp.tile([HID, 256], F32, tag="hps", name="hps")
                        ef_t = pool.tile([EDGE_DIM + 1, 256], FP16, tag="ef", name="ef")
                        nc.sync.dma_start(
                            ef_t[:, 0:ncols],
                            ef_d[:, t * 128 : t * 128 + ncols],
                        )
                        nc.tensor.matmul(
                            h_ps[:, 0:ncols], w1_sb[:], ef_t[:, 0:ncols],
                            start=True, stop=True,
                        )
                        nc.scalar.activation(
                            hb[0:HID, 0:ncols], h_ps[:, 0:ncols], AF.Relu
                        )
                    pk_t = pool.tile([128, PKW], FP16, tag="pk", name="pk")
                    nc.sync.dma_start(pk_t[:], pk_d[t])

                    # --- MLP2: rw = h @ W2 (fp32r, 2 psum banks) ---
                    rw_lo = rp.tile([128, 384], F32, tag="rwlo", name="rwlo")
                    nc.tensor.matmul(
                        rw_lo[:], hb[:, hoff : hoff + 128], w2_sb[:, 0:384],
                        start=True, stop=True,
                    )
                    rw_hi = rp.tile([128, 384], F32, tag="rwhi", name="rwhi")
                    nc.tensor.matmul(
                        rw_hi[:], hb[:, hoff : hoff + 128], w2_sb[:, 384:768],
                        start=True, stop=True,
                    )
                    rw_bf = pool.tile([128, 768], FP16, tag="rwbf", name="rwbf")
                    nc.scalar.activation(rw_bf[:, 0:384], rw_lo[:], AF.Copy)
                    nc.scalar.activation(rw_bf[:, 384:768], rw_hi[:], AF.Copy)

                    # --- tmp[d, m] = sum_k fe[m1,k]*basis[k,(r,d)]  (bf16) ---
                    prod = pool.tile([128, 288], FP16, tag="prod", name="prod")
                    fe_v = (
                        pk_t[:, 0:48]
                        .rearrange("p (m k) -> p m k", k=3)
                        .unsqueeze(1)
                        .broadcast_to([128, 3, M1, 3])
                    )
                    pv = prod[:].rearrange("p (d m r k) -> p d m r k", m=M1, r=2, k=3)
                    for r_i in range(2):
                        bas_r = (
                            pk_t[:, 48:66]
                            .rearrange("p (d r k) -> p d r k", r=2, k=3)[:, :, r_i]
                            .unsqueeze(2)
                            .broadcast_to([128, 3, M1, 3])
                        )
                        PRODENG.tensor_mul(pv[:, :, :, r_i], fe_v, bas_r)
                    tmp_t = pool.tile([128, 96], FP16, tag="tmp", name="tmp")
                    tmv = tmp_t[:].rearrange("p (d m) -> p d m", d=3)
                    pvk = pv.rearrange("p d m r k -> p d (m r) k")
                    nc.vector.tensor_add(tmv, pvk[:, :, :, 0], pvk[:, :, :, 1])
                    nc.vector.tensor_add(tmv, tmv, pvk[:, :, :, 2])

                    # --- pc[c, d, m] = rw[c, m] * tmp[d, m]  (bf16 2x) ---
                    # channels [0, CPOOL) on Pool, rest on DVE
                    pc = pool.tile([128, 2304], FP16, tag="pc", name="pc")
                    pcv = pc[:].rearrange("p (c d m) -> p c d m", d=3, m=32)
                    rw_v = (
                        rw_bf[:]
                        .rearrange("p (c m) -> p c m", m=32)
                        .unsqueeze(2)
                        .broadcast_to([128, 24, 3, 32])
                    )
                    tmp_v = (
                        tmp_t[:]
                        .rearrange("p (d m) -> p d m", m=32)
                        .unsqueeze(1)
                        .broadcast_to([128, 24, 3, 32])
                    )
                    if CPOOL:
                        nc.gpsimd.tensor_mul(
                            pcv[:, 0:CPOOL], rw_v[:, 0:CPOOL], tmp_v[:, 0:CPOOL]
                        )
                    nc.vector.tensor_mul(
                        pcv[:, CPOOL:24], rw_v[:, CPOOL:24], tmp_v[:, CPOOL:24]
                    )

                    # --- k,q m-reduction: bf16 trees (Pool slice + DVE slice) ---
                    kqred = pool.tile([128, 48], FP16, tag="kqred", name="kqred")
                    for eng, c0, c1 in (
                        (nc.gpsimd, 0, CPOOL),
                        (nc.vector, CPOOL, CKQ),
                    ):
                        if c0 == c1:
                            continue
                        tv = pcv[:, c0:c1]  # [p, C, 3, 32] in-place tree
                        for half in (16, 8, 4, 2):
                            eng.tensor_add(
                                tv[:, :, :, 0:half], tv[:, :, :, 0:half],
                                tv[:, :, :, half : 2 * half],
                            )
                        eng.tensor_add(
                            kqred[:, 3 * c0 : 3 * c1].rearrange(
                                "p (c d) -> p c d", d=3
                            ),
                            tv[:, :, :, 0], tv[:, :, :, 1],
                        )

                    # --- scores -> leaky -> exp (temp folded into W2) ---
                    p4 = pool.tile([128, 24], FP16, tag="p4", name="p4")
                    nc.vector.tensor_mul(p4[:], kqred[:, 0:24], kqred[:, 24:48])
                    s4 = pool.tile([128, 4], F32, tag="s4", name="s4")
                    nc.vector.tensor_reduce(
                        s4[:], p4[:].rearrange("p (h j) -> p h j", j=6),
                        axis=AX.X, op=ALU.add,
                    )
                    l4 = pool.tile([128, 4], F32, tag="l4", name="l4")
                    nc.vector.scalar_tensor_tensor(
                        l4[:], s4[:], 0.2, s4[:], ALU.mult, ALU.max
                    )
                    ex = pool.tile([128, 4], F32, tag="ex", name="ex")
                    nc.scalar.activation(ex[:], l4[:], AF.Exp, bias=nbias[:])

                    # --- fused one-hot * ex; seg matmuls (v m-sum via psum) ---
                    nidf = pool.tile([128, 1], F32, tag="nidf", name="nidf")
                    nc.vector.tensor_copy(nidf[:], pk_t[:, 66:67])
                    for h in range(NHEADS):
                        oh = pool.tile([128, 128], FP16, tag=f"oh{h}", name=f"oh{h}")
                        nc.vector.tensor_scalar(
                            oh[:], io_sb[:], nidf[:], ex[:, h : h + 1],
                            ALU.is_equal, ALU.mult,
                        )
                        segp = segs[h // 2]
                        hh = h % 2
                        nc.tensor.matmul(
                            segp[:, hh * 193 : hh * 193 + 1], oh[:], ones_sb[:],
                            start=False, stop=(t == t1 - 1),
                            skip_group_check=True,
                        )
                        nc.tensor.matmul(
                            segp[:, hh * 193 + 1 : hh * 193 + 193], oh[:],
                            pc[:, 1536 + h * 192 : 1536 + (h + 1) * 192],
                            start=False, stop=(t == t1 - 1),
                            skip_group_check=True,
                        )

                # --- window flush: num/den, m-reduce on Pool, divide ---
                outf = sm.tile([128, HIDDEN], F32, tag="outf", name="outf")
                rcp = sm.tile([128, 4], F32, tag="rcp", name="rcp")
                den = sm.tile([128, 4], F32, tag="den", name="den")
                for p_i in range(2):
                    nc.vector.tensor_scalar(
                        den[:, p_i * 2 : p_i * 2 + 2],
                        segs[p_i][:].rearrange("p (a b) -> p a b", a=2)[:, :, 0],
                        1e-30, None, ALU.add,
                    )
                nc.vector.reciprocal(rcp[:], den[:])
                for p_i in range(2):
                    nump = sm.tile([128, 12], F32, tag=f"num{p_i}", name=f"num{p_i}")
                    for hh in range(2):
                        nc.vector.tensor_reduce(
                            nump[:, hh * 6 : hh * 6 + 6].rearrange(
                                "p (c d) -> p c d", c=2
                            ),
                            segs[p_i][:, hh * 193 + 1 : hh * 193 + 193].rearrange(
                                "p (c d m) -> p c d m", c=2, m=32
                            ),
                            axis=AX.X, op=ALU.add,
                        )
                    nc.vector.tensor_mul(
                        outf[:, p_i * 12 : (p_i + 1) * 12].rearrange(
                            "p (h j) -> p h j", h=2
                        ),
                        nump[:].rearrange("p (h j) -> p h j", h=2),
                        rcp[:, p_i * 2 : p_i * 2 + 2].unsqueeze(2).broadcast_to(
                            [128, 2, 6]
                        ),
                    )
                nc.sync.dma_start(out_d[w * 128 : (w + 1) * 128, :], outf[:])
    nc.finalize()
    return nc


def _prep(src, dst, basis, edge_feats, f, W1, b1, W2, b2):
    src = np.asarray(src).astype(np.int64)
    dst = np.asarray(dst).astype(np.int64)
    basis = np.asarray(basis, dtype=np.float32)
    edge_feats = np.asarray(edge_feats, dtype=np.float32)
    f = np.asarray(f, dtype=np.float32)

    order = np.argsort(dst, kind="stable")
    ds = dst[order]
    # global windows of 128 nodes
    wstart = np.arange(0, N, WIN)
    wcuts = np.searchsorted(ds, np.append(wstart, N))
    wcnt = wcuts[1:] - wcuts[:-1]                      # edges per window
    # snake-deal windows (by size desc) into cores to balance slot maxima
    nwin = (NWIN_G + NC - 1) // NC                     # slots per core
    order_w = np.argsort(-wcnt, kind="stable")
    assign = [[] for _ in range(NC)]
    for i, wi in enumerate(order_w):
        rnd, pos = divmod(i, NC)
        c = pos if rnd % 2 == 0 else NC - 1 - pos
        assign[c].append(int(wi))
    for c in range(NC):
        while len(assign[c]) < nwin:
            assign[c].append(-1)                       # empty slot
    # per-slot tile counts = max over cores
    tw = np.zeros(nwin, dtype=np.int64)
    for s in range(nwin):
        m = 1
        for c in range(NC):
            wi = assign[c][s]
            if wi >= 0:
                m = max(m, (int(wcnt[wi]) + 127) // 128)
        tw[s] = m
    toff = np.zeros(nwin + 1, dtype=np.int64)
    toff[1:] = np.cumsum(tw)
    T = int(toff[-1])
    EP = T * 128

    s_sc = np.ones(768, dtype=np.float32)
    s_sc[: CKQ * 32] = TEMP**0.5                       # k,q carry sqrt(temp)
    w1_aug = np.concatenate(
        [np.asarray(W1, dtype=np.float32).T, np.asarray(b1, dtype=np.float32)[None, :]]
    )
    w2_aug = np.concatenate(
        [
            np.asarray(W2, dtype=np.float32).T * s_sc[None, :],
            (np.asarray(b2, dtype=np.float32) * s_sc)[None, :],
        ]
    )
    iota = np.broadcast_to(np.arange(128, dtype=np.float32)[None, :], (128, 128))

    in_maps = []
    for c in range(NC):
        efT = np.zeros((EDGE_DIM + 1, EP), dtype=np.float32)  # cast at end
        packed = np.zeros((T, 128, PKW), dtype=np.float32)
        packed[:, :, 66] = -1.0
        packed[:, :, 67] = -1.0
        for s in range(nwin):
            wi = assign[c][s]
            if wi < 0:
                continue
            a, b = wcuts[wi], wcuts[wi + 1]
            idx = order[a:b]
            k = len(idx)
            if k == 0:
                continue
            base = int(toff[s]) * 128
            efT[:EDGE_DIM, base : base + k] = edge_feats[idx].T
            efT[EDGE_DIM, base : base + k] = 1.0
            flat = packed.reshape(T * 128, PKW)
            flat[base : base + k, 0:48] = f[src[idx]].reshape(k, 48)
            # basis [k, d1=3(kk), (r,d)=6] -> basT [d, r, kk]
            bt = basis[idx].reshape(k, 3, 2, 3)        # (kk, r, d)
            flat[base : base + k, 48:66] = bt.transpose(0, 3, 2, 1).reshape(k, 18)
            flat[base : base + k, 66] = (dst[idx] - wi * WIN).astype(np.float32)
        in_maps.append(
            {
                "efT": efT.astype(np.float16),
                "packed": packed.astype(np.float16),
                "w1": w1_aug.astype(np.float16),
                "w2": w2_aug.astype(np.float16),
                "iota": iota.astype(np.float16),
            }
        )
    return T, toff, nwin, assign, in_maps


def kernel(src, dst, basis, edge_feats, f, W1, b1, W2, b2):
    global LAST_RUN_S, LAST_RESULTS
    T, toff, nwin, assign, in_maps = _prep(
        src, dst, basis, edge_feats, f, W1, b1, W2, b2
    )
    key = (T, tuple(toff), nwin)
    if key not in _CACHE:
        _CACHE[key] = _build(T, toff, nwin)
    nc = _CACHE[key]
    t0 = time.time()
    import os

    trace = bool(os.environ.get("BASS_KTRACE"))
    res = run_bass_kernel_spmd(nc, in_maps, list(range(NC)), trace=trace)
    LAST_RUN_S = time.time() - t0
    LAST_RESULTS = res
    out = np.zeros((N + WIN, HIDDEN), dtype=np.float32)
    for c in range(NC):
        oc = res.results[c]["out"]
        for s in range(nwin):
            wi = assign[c][s]
            if wi < 0:
                continue
            n0 = wi * WIN
            n1 = min(n0 + WIN, N)
            out[n0:n1] = oc[s * 128 : s * 128 + (n1 - n0)]
    return out[:N].reshape(N, M2, D2)
